# revision 36
# baseline (speedup 1.0000x reference)
"""Trainium2 Bass kernel for AttentionBlock (B=8, C=512, H=W=32, 8 heads, GN-32).

Strategy: pure data-parallel over batch — one batch element per NeuronCore,
no collectives. Per core:
  - GroupNorm via bn_stats/bn_aggr + PE indicator-matmul group aggregation.
  - qkv: q,k in [c,t] layout (lhsT = Wqk^T), v^T in [t,c] layout
    (lhsT = xn), with bias folded in via an augmented K=513 contraction.
  - Attention per head pair: S^T = k^T q on PE with BOTH heads running
    concurrently in the two PE row halves (tile_position (64,0) for the
    odd head — ~3x faster than serial half-K matmuls), exp on ACT
    (scale=1/8, no max subtraction — S/8 ~ N(0,1)), AV with a ones column
    in v^T producing the softmax denominator for free.
  - Normalize off the PE critical path: one DVE evac of the PSUM AV
    output, reciprocal_approx_fast on the denominator row (base 0),
    gpsimd partition_broadcast, DVE multiply.
  - v and the later heads' q,k matmuls are fed INTO the attention stream
    as a rate-controlled background generator, so ACT exp work starts as
    early as possible (ACT exp at 1.2 GHz is the steady-state limit).
  - proj with pre-transposed pw/sqrt(2) and augmented-K bias, residual
    add against host-prescaled x/sqrt(2); PSUM from the shared aux pool
    (no pool-transition barrier), nt-outer so the first half overlaps the
    last pair's normalize.
Host side prepares transposed/permuted bf16 weights; matmuls run bf16.
"""

import math
from contextlib import ExitStack

import numpy as np
import ml_dtypes

import concourse.bacc as bacc
import concourse.bass as bass
import concourse.tile as tile
import concourse.mybir as mybir
from concourse.bass_utils import run_bass_kernel_spmd

B, C, T = 8, 512, 1024
NH, CH = 8, 64
NG, GSZ = 32, 16  # groups, channels per group
EPS = 1e-6
N_CORES = 8

F32 = mybir.dt.float32
BF16 = mybir.dt.bfloat16
AF = mybir.ActivationFunctionType
OP = mybir.AluOpType

_CACHED = {}


def _indicator_consts():
    # gfwd_p[p, kt*NG + g] = 1/16 if channel kt*128+p in group g — already in
    # SBUF partition-major layout so the DMA is contiguous (128 descriptors)
    gfwd_p = np.zeros((128, 4 * NG), np.float32)
    for c in range(C):
        kt, p = divmod(c, 128)
        gfwd_p[p, kt * NG + c // GSZ] = 1.0 / GSZ
    # gbck[g, c] = 1 if channel c in group g
    gbck = np.zeros((NG, C), np.float32)
    for c in range(C):
        gbck[c // GSZ, c] = 1.0
    return gfwd_p, gbck


def build_graph():
    nc = bacc.Bacc("TRN2", target_bir_lowering=False, debug=False,
                   num_devices=N_CORES)

    x_ext = nc.dram_tensor("x", [C, T], F32, kind="ExternalInput")
    wqk_ext = nc.dram_tensor("wqk", [C, 2 * C], BF16, kind="ExternalInput")
    qkb_ext = nc.dram_tensor("qkb", [128, 8], F32, kind="ExternalInput")
    wv_ext = nc.dram_tensor("wv", [C + 1, C], BF16, kind="ExternalInput")
    pw_ext = nc.dram_tensor("pw", [C, C], BF16, kind="ExternalInput")
    pb_ext = nc.dram_tensor("pb", [128, 4], F32, kind="ExternalInput")
    gnw_ext = nc.dram_tensor("gnw", [128, 4], F32, kind="ExternalInput")
    gnb_ext = nc.dram_tensor("gnb", [128, 4], F32, kind="ExternalInput")
    out_ext = nc.dram_tensor("out", [C, T], BF16, kind="ExternalOutput")

    gfwd_np, gbck_np = _indicator_consts()
    gfwd_dram = nc.inline_tensor(gfwd_np, "gfwd")
    gbck_dram = nc.inline_tensor(gbck_np, "gbck")

    with tile.TileContext(nc) as tc, ExitStack() as ctx:
        pers = ctx.enter_context(tc.tile_pool(name="pers", bufs=1))
        work = ctx.enter_context(tc.tile_pool(name="work", bufs=2))
        small = ctx.enter_context(tc.tile_pool(name="small", bufs=2))
        p_pool = ctx.enter_context(tc.tile_pool(name="pT", bufs=4))
        rb_pool = ctx.enter_context(tc.tile_pool(name="rb", bufs=2))
        o_pool = ctx.enter_context(tc.tile_pool(name="osb", bufs=4))

        # ---- persistent SBUF tensors ----
        x_sb = [pers.tile([128, T], F32, tag=f"x{i}", name=f"x{i}") for i in range(4)]
        xn_sb = [pers.tile([128, T], BF16, tag=f"xn{i}", name=f"xn{i}") for i in range(4)]
        aun_sb = [pers.tile([64, 512], F32, tag=f"au{i}", name=f"au{i}")
                  for i in range(16)]  # unnormalized AV, (pr, h%2, nt)
        den_sb = [pers.tile([1, 512], F32, tag=f"de{i}", name=f"de{i}")
                  for i in range(16)]  # softmax denominators at base partition 0
        qk_sb = [pers.tile([128, T], BF16, tag=f"qk{i}", name=f"qk{i}") for i in range(8)]
        v_sb = [pers.tile([128, 520], BF16, tag=f"v{i}", name=f"v{i}") for i in range(8)]
        a_sb = [pers.tile([128, T], BF16, tag=f"a{i}", name=f"a{i}") for i in range(4)]
        wqk_sb = [pers.tile([128, 2 * C], BF16, tag=f"wqk{i}", name=f"wqk{i}") for i in range(4)]
        wv_sb = [pers.tile([128, C], BF16, tag=f"wv{i}", name=f"wv{i}") for i in range(4)]
        wv4_sb = pers.tile([1, C], BF16, tag="wv4", name="wv4")
        pw_sb = [pers.tile([128, C], BF16, tag=f"pw{i}", name=f"pw{i}") for i in range(4)]
        pb_sb = pers.tile([128, 4], F32, tag="pb", name="pb")
        qkb_sb = pers.tile([128, 8], F32, tag="qkb", name="qkb")
        gnw_sb = pers.tile([128, 4], F32, tag="gnw", name="gnw")
        gnb_sb = pers.tile([128, 4], F32, tag="gnb", name="gnb")
        gfwd_sb = pers.tile([128, 4 * NG], F32, tag="gfwd", name="gfwd")
        gbck_sb = pers.tile([NG, C], F32, tag="gbck", name="gbck")
        ones_sb = pers.tile([1, T], BF16, tag="ones", name="ones")
        gst_sb = pers.tile([NG, 2], F32, tag="gst", name="gst")
        ab_sb = [pers.tile([128, 2], F32, tag=f"ab{i}", name=f"ab{i}") for i in range(4)]

        # ---- input DMAs, split across the two HWDGE queues (SP + ACT) ----
        nc.sync.dma_start(x_sb[0][:], x_ext.ap()[0:128, :])
        nc.scalar.dma_start(x_sb[1][:], x_ext.ap()[128:256, :])
        nc.sync.dma_start(x_sb[2][:], x_ext.ap()[256:384, :])
        nc.scalar.dma_start(x_sb[3][:], x_ext.ap()[384:512, :])
        nc.sync.dma_start(gfwd_sb[:], gfwd_dram.ap())
        nc.sync.dma_start(gbck_sb[:], gbck_dram.ap())
        nc.sync.dma_start(gnw_sb[:], gnw_ext.ap())
        nc.sync.dma_start(gnb_sb[:], gnb_ext.ap())
        nc.sync.dma_start(qkb_sb[:], qkb_ext.ap())
        for i in [0, 2]:
            nc.sync.dma_start(wqk_sb[i][:], wqk_ext.ap()[128 * i:128 * (i + 1), :])
        for i in [1, 3]:
            nc.sync.dma_start(wv_sb[i][:], wv_ext.ap()[128 * i:128 * (i + 1), :])
        nc.sync.dma_start(wv4_sb[:], wv_ext.ap()[C:C + 1, :])
        nc.vector.memset(ones_sb[:], 1.0)

        # ---- GroupNorm statistics (ACT accumulate — DVE stays free and the
        # stats sit at the HEAD of ACT's queue, before weight DMA issues) ----
        with tc.tile_pool(name="ps_misc", bufs=1, space="PSUM") as ps_misc:
            ps_g = ps_misc.tile([NG, 2], F32, tag="g", name="g")
            for i in range(4):
                # per-channel mean and E[x^2] via ACT free-axis accumulation;
                # Copy/Square live in every ACT table set — no table switch
                scr = work.tile([128, T], F32, tag="scr", name="scr")
                st2 = small.tile([128, 2], F32, tag="st2", name="st2")
                nc.scalar.activation(scr[:], x_sb[i][:], AF.Copy,
                                     scale=1.0 / T, accum_out=st2[:, 0:1])
                nc.scalar.activation(scr[:], x_sb[i][:], AF.Square,
                                     scale=1.0 / 32.0, accum_out=st2[:, 1:2])
                nc.tensor.matmul(ps_g[:, :], gfwd_sb[:, NG * i:NG * (i + 1)],
                                 st2[:, :], start=(i == 0), stop=(i == 3))

            # late weight DMAs: issued from ACT behind the stats, plus sync
            for i in [1, 3]:
                nc.scalar.dma_start(wqk_sb[i][:],
                                    wqk_ext.ap()[128 * i:128 * (i + 1), :])
            for i in [0, 2]:
                nc.scalar.dma_start(wv_sb[i][:],
                                    wv_ext.ap()[128 * i:128 * (i + 1), :])
            nc.scalar.dma_start(pb_sb[:], pb_ext.ap())
            for i in range(4):
                eng = nc.sync if i % 2 == 0 else nc.scalar
                eng.dma_start(pw_sb[i][:], pw_ext.ap()[128 * i:128 * (i + 1), :])

            # group var = E[x^2] - mean^2 ;  grs = rsqrt(var + eps)
            gsb = small.tile([NG, 2], F32, tag="gsb", name="gsb")
            nc.vector.tensor_copy(gsb[:], ps_g[:, :])
            gvar = small.tile([NG, 1], F32, tag="gvar", name="gvar")
            nc.vector.scalar_tensor_tensor(
                gvar[:], gsb[:, 0:1], gsb[:, 0:1], gsb[:, 1:2],
                op0=OP.mult, op1=OP.subtract)  # mean^2 - E[x^2] = -var
            nc.vector.tensor_scalar(gvar[:], gvar[:], -1.0, EPS,
                                    op0=OP.mult, op1=OP.add)  # var + eps
            r = small.tile([NG, 1], F32, tag="r", name="r")
            nc.vector.reciprocal(r[:], gvar[:])
            y = small.tile([NG, 1], F32, tag="y", name="y")
            nc.vector.tensor_scalar(y[:], r[:], 0.5, 0.5, op0=OP.mult, op1=OP.add)
            for _ in range(3):  # Newton for sqrt(r): y = 0.5*(y + r/y)
                q = small.tile([NG, 1], F32, tag="q", name="q")
                nc.vector.reciprocal(q[:], y[:])
                t = small.tile([NG, 1], F32, tag="t", name="t")
                nc.vector.tensor_mul(t[:], r[:], q[:])
                y2 = small.tile([NG, 1], F32, tag="y", name="y")
                nc.vector.tensor_add(y2[:], y[:], t[:])
                nc.vector.tensor_scalar(y2[:], y2[:], 0.5, None, op0=OP.mult)
                y = y2
            nc.vector.tensor_copy(gst_sb[:, 0:1], gsb[:, 0:1])
            nc.vector.tensor_copy(gst_sb[:, 1:2], y[:])

        with tc.tile_pool(name="ps_bc", bufs=2, space="PSUM") as ps_bcp:
            for i in range(4):
                ps_bc = ps_bcp.tile([128, 2], F32, tag="bc", name="bc")
                nc.tensor.matmul(ps_bc[:, :], gbck_sb[:, 128 * i:128 * (i + 1)],
                                 gst_sb[:, :], start=True, stop=True)
                # A = gnw * grs ; B = gnb - gmean * A
                nc.vector.tensor_mul(ab_sb[i][:, 0:1], gnw_sb[:, i:i + 1],
                                     ps_bc[:, 1:2])
                tmp = small.tile([128, 1], F32, tag="tmp", name="tmp")
                nc.vector.tensor_mul(tmp[:], ps_bc[:, 0:1], ab_sb[i][:, 0:1])
                nc.vector.tensor_sub(ab_sb[i][:, 1:2], gnb_sb[:, i:i + 1], tmp[:])
                # xn = x*A + B   (bf16)
                nc.vector.tensor_scalar(xn_sb[i][:], x_sb[i][:],
                                        ab_sb[i][:, 0:1], ab_sb[i][:, 1:2],
                                        op0=OP.mult, op1=OP.add)

        # ---- attention + background-fed v / qk ----
        with tc.tile_pool(name="ps_aux", bufs=2, space="PSUM") as ps_auxp, \
             tc.tile_pool(name="ps_s", bufs=2, space="PSUM") as ps_sp, \
             tc.tile_pool(name="ps_a", bufs=1, space="PSUM") as ps_ap:

            def v_gen(mt):
                # v^T chunk [t=128, c] with ones column -> v_sb[mt]
                ps_v = ps_auxp.tile([128, C], F32, tag="aux", name="ps_v")
                for kt in range(4):
                    nc.tensor.matmul(ps_v[:, :],
                                     xn_sb[kt][:, 128 * mt:128 * (mt + 1)],
                                     wv_sb[kt][:, :], start=(kt == 0), stop=False)
                    yield
                nc.tensor.matmul(ps_v[:, :], ones_sb[:, 0:128], wv4_sb[:, :],
                                 start=False, stop=True)
                nc.vector.tensor_copy(
                    v_sb[mt][:].rearrange("p (h w) -> p h w", w=CH + 1)[:, :, 0:CH],
                    ps_v[:].rearrange("p (h w) -> p h w", w=CH))
                nc.gpsimd.memset(
                    v_sb[mt][:].rearrange("p (h w) -> p h w", w=CH + 1)[:, :, CH:CH + 1],
                    1.0)
                yield

            def qk_gen(mt):
                # q or k row-block [128, T], bias added
                for nt in range(2):
                    ps_qk = ps_auxp.tile([128, 512], F32, tag="aux", name="ps_qk")
                    for kt in range(4):
                        nc.tensor.matmul(ps_qk[:, :],
                                         wqk_sb[kt][:, 128 * mt:128 * (mt + 1)],
                                         xn_sb[kt][:, 512 * nt:512 * (nt + 1)],
                                         start=(kt == 0), stop=(kt == 3))
                        yield
                    nc.vector.tensor_scalar(qk_sb[mt][:, 512 * nt:512 * (nt + 1)],
                                            ps_qk[:, :],
                                            qkb_sb[:, mt:mt + 1], None, op0=OP.add)
                    yield

            def emit_evac(h, ps_a, nt):
                # evacuate unnormalized AV + denom row to SBUF — frees the
                # PSUM bank fast so the next AV group isn't blocked. Denom
                # goes to base partition 0 (reciprocal_approx_fast needs it).
                idx = 4 * (h // 2) + 2 * (h % 2) + nt
                nc.vector.tensor_copy(aun_sb[idx][:], ps_a[0:CH, :])
                nc.vector.tensor_copy(den_sb[idx][:], ps_a[CH:CH + 1, :])

            def emit_norm(h, nt):
                # off-critical-path normalize: fast reciprocal, gpsimd
                # partition broadcast, DVE multiply
                idx = 4 * (h // 2) + 2 * (h % 2) + nt
                recip = small.tile([1, 512], F32, tag="recip", name="recip")
                nc.vector.reciprocal_approx_fast(recip[:], den_sb[idx][:])
                rb = rb_pool.tile([CH, 512], F32, tag="rb", name="rb")
                nc.gpsimd.partition_broadcast(rb[:], recip[:])
                dst = a_sb[h // 2][64 * (h % 2):64 * (h % 2) + 64,
                                   512 * nt:512 * (nt + 1)]
                nc.vector.tensor_mul(dst, aun_sb[idx][:], rb[:])

            # background feed: v first (AV of pair 0 needs it), then q,k for
            # pairs 1-3. Consumption rate decays as attention reaches ACT-bound
            # steady state.
            import itertools
            bg = itertools.chain(
                *[v_gen(mt) for mt in range(8)],
                qk_gen(1), qk_gen(5), qk_gen(2), qk_gen(6),
                qk_gen(3), qk_gen(7))

            for _ in qk_gen(0):
                pass
            for _ in qk_gen(4):
                pass

            pending = [None]  # deferred AV emitter (software pipeline depth 1)

            def flush():
                if pending[0] is not None:
                    pending[0]()
                    pending[0] = None

            gstep = 0
            for pr in range(4):
                hA, hB = 2 * pr, 2 * pr + 1
                qA, kA = qk_sb[pr], qk_sb[4 + pr]
                for nt in range(2):
                    ps_aA = ps_ap.tile([128, 512], F32, tag="aA", name="aA")
                    ps_aB = ps_ap.tile([128, 512], F32, tag="aB", name="aB")
                    for st in range(8):
                        ps_s = ps_sp.tile([128, T], F32, tag="s", name="s")
                        # two heads run CONCURRENTLY in the PE row halves:
                        # head A in rows 0-63, head B in rows 64-127 via
                        # tile_position (64, 0) — ~3x faster than serial.
                        nc.tensor.matmul(ps_s[:, 0:512],
                                         kA[0:64, 128 * st:128 * (st + 1)],
                                         qA[0:64, 512 * nt:512 * (nt + 1)],
                                         start=True, stop=True)
                        nc.tensor.matmul(ps_s[:, 512:1024],
                                         kA[64:128, 128 * st:128 * (st + 1)],
                                         qA[64:128, 512 * nt:512 * (nt + 1)],
                                         start=True, stop=True,
                                         tile_position=(64, 0))
                        pT = p_pool.tile([128, T], BF16, tag="pT", name="pT")
                        nc.scalar.activation(pT[:], ps_s[:], AF.Exp, scale=0.125)
                        flush()
                        rate = 5 if gstep < 8 else (3 if gstep < 16 else 2)
                        for _ in range(rate):
                            next(bg, None)
                        gstep += 1

                        def mk(aA=ps_aA, aB=ps_aB, p=pT, s=st, hA=hA, hB=hB,
                               nt=nt):
                            def emit():
                                nc.tensor.matmul(
                                    aA[0:CH + 1, :],
                                    v_sb[s][:, (CH + 1) * hA:(CH + 1) * (hA + 1)],
                                    p[:, 0:512],
                                    start=(s == 0), stop=(s == 7))
                                nc.tensor.matmul(
                                    aB[0:CH + 1, :],
                                    v_sb[s][:, (CH + 1) * hB:(CH + 1) * (hB + 1)],
                                    p[:, 512:1024],
                                    start=(s == 0), stop=(s == 7))
                                if s == 7:
                                    emit_evac(hA, aA, nt)
                                    emit_evac(hB, aB, nt)
                                    emit_norm(hA, nt)
                                    emit_norm(hB, nt)
                            return emit

                        pending[0] = mk()
            flush()
            for _ in bg:  # any feed leftovers
                pass

            # ---- proj + residual, reusing the aux PSUM pool (no pool
            # transition barrier between attention and proj). nt outer so the
            # nt=0 half overlaps the last pair's nt=1 normalize chain. ----
            for nt in range(2):
                for ot in range(4):
                    ps_h = ps_auxp.tile([128, C], F32, tag="aux", name="ps_h")
                    for kt in range(4):
                        nc.tensor.matmul(ps_h[:, :],
                                         pw_sb[kt][:, 128 * ot:128 * (ot + 1)],
                                         a_sb[kt][:, 512 * nt:512 * (nt + 1)],
                                         start=(kt == 0), stop=(kt == 3))
                    out_t = o_pool.tile([128, 512], BF16, tag="osb", name="osb")
                    nc.vector.scalar_tensor_tensor(
                        out_t[:], ps_h[:, :], pb_sb[:, ot:ot + 1],
                        x_sb[ot][:, 512 * nt:512 * (nt + 1)],
                        op0=OP.add, op1=OP.add)
                    eng = nc.sync if (2 * ot + nt) % 2 == 0 else nc.scalar
                    eng.dma_start(
                        out_ext.ap()[128 * ot:128 * (ot + 1),
                                     512 * nt:512 * (nt + 1)],
                        out_t[:])

    nc.compile()
    return nc


def _prep_host(x, gn_w, gn_b, qkv_w, qkv_b, proj_w, proj_b):
    s2 = 1.0 / math.sqrt(2.0)
    x2 = (x.reshape(B, C, T) * s2).astype(np.float32)

    w3 = np.asarray(qkv_w, np.float32).reshape(NH, 3, CH, C)
    b3 = np.asarray(qkv_b, np.float32).reshape(NH, 3, CH)
    qw = w3[:, 0].reshape(C, C)
    kw = w3[:, 1].reshape(C, C)
    vw = w3[:, 2].reshape(C, C)
    qb = b3[:, 0].reshape(C)
    kb = b3[:, 1].reshape(C)
    vb = b3[:, 2].reshape(C)

    wqk = np.concatenate([qw, kw], 0).T.astype(ml_dtypes.bfloat16)  # [512, 1024]
    qkb = np.concatenate([qb, kb]).reshape(8, 128).T.astype(np.float32).copy()
    wv = np.concatenate([vw.T, vb[None, :]], 0).astype(ml_dtypes.bfloat16)
    pw = (np.asarray(proj_w, np.float32).T * s2).astype(ml_dtypes.bfloat16)
    pb = (np.asarray(proj_b, np.float32) * s2).reshape(4, 128).T.copy()
    gnw_t = np.asarray(gn_w, np.float32).reshape(4, 128).T.copy()
    gnb_t = np.asarray(gn_b, np.float32).reshape(4, 128).T.copy()

    base = {"wqk": np.ascontiguousarray(wqk), "qkb": qkb,
            "wv": np.ascontiguousarray(wv), "pw": np.ascontiguousarray(pw),
            "pb": pb, "gnw": gnw_t, "gnb": gnb_t}
    return x2, base


def kernel(x, gn_w, gn_b, qkv_w, qkv_b, proj_w, proj_b):
    x2, base = _prep_host(x, gn_w, gn_b, qkv_w, qkv_b, proj_w, proj_b)
    if "nc" not in _CACHED:
        _CACHED["nc"] = build_graph()
    nc = _CACHED["nc"]
    in_maps = [dict(base, x=np.ascontiguousarray(x2[i])) for i in range(N_CORES)]
    res = run_bass_kernel_spmd(nc, in_maps, core_ids=list(range(N_CORES)))
    out = np.stack([res.results[i]["out"].astype(np.float32)
                    for i in range(N_CORES)], 0)
    return out.reshape(B, C, 32, 32)


# revision 37
# speedup vs baseline: 1.0195x; 1.0195x over previous
"""Trainium2 Bass kernel for AttentionBlock (B=8, C=512, H=W=32, 8 heads, GN-32).

Strategy: pure data-parallel over batch — one batch element per NeuronCore,
no collectives. Per core:
  - GroupNorm via bn_stats/bn_aggr + PE indicator-matmul group aggregation.
  - qkv: q,k in [c,t] layout (lhsT = Wqk^T), v^T in [t,c] layout
    (lhsT = xn), with bias folded in via an augmented K=513 contraction.
  - Attention per head pair: S^T = k^T q on PE with BOTH heads running
    concurrently in the two PE row halves (tile_position (64,0) for the
    odd head — ~3x faster than serial half-K matmuls), exp on ACT
    (scale=1/8, no max subtraction — S/8 ~ N(0,1)), AV with a ones column
    in v^T producing the softmax denominator for free.
  - Normalize off the PE critical path: one DVE evac of the PSUM AV
    output, reciprocal_approx_fast on the denominator row (base 0),
    gpsimd partition_broadcast, DVE multiply.
  - v and the later heads' q,k matmuls are fed INTO the attention stream
    as a rate-controlled background generator, so ACT exp work starts as
    early as possible (ACT exp at 1.2 GHz is the steady-state limit).
  - proj with pre-transposed pw/sqrt(2) and augmented-K bias, residual
    add against host-prescaled x/sqrt(2); PSUM from the shared aux pool
    (no pool-transition barrier), nt-outer so the first half overlaps the
    last pair's normalize.
Host side prepares transposed/permuted bf16 weights; matmuls run bf16.
"""

import math
from contextlib import ExitStack

import numpy as np
import ml_dtypes

import concourse.bacc as bacc
import concourse.bass as bass
import concourse.tile as tile
import concourse.mybir as mybir
from concourse.bass_utils import run_bass_kernel_spmd

B, C, T = 8, 512, 1024
NH, CH = 8, 64
NG, GSZ = 32, 16  # groups, channels per group
EPS = 1e-6
N_CORES = 8

F32 = mybir.dt.float32
BF16 = mybir.dt.bfloat16
AF = mybir.ActivationFunctionType
OP = mybir.AluOpType

_CACHED = {}


def _indicator_consts():
    # gfwd_p[p, kt*NG + g] = 1/16 if channel kt*128+p in group g — already in
    # SBUF partition-major layout so the DMA is contiguous (128 descriptors)
    gfwd_p = np.zeros((128, 4 * NG), np.float32)
    for c in range(C):
        kt, p = divmod(c, 128)
        gfwd_p[p, kt * NG + c // GSZ] = 1.0 / GSZ
    # gbck[g, c] = 1 if channel c in group g
    gbck = np.zeros((NG, C), np.float32)
    for c in range(C):
        gbck[c // GSZ, c] = 1.0
    return gfwd_p, gbck


def build_graph():
    nc = bacc.Bacc("TRN2", target_bir_lowering=False, debug=False,
                   num_devices=N_CORES)

    x_ext = nc.dram_tensor("x", [C, T], F32, kind="ExternalInput")
    wqk_ext = nc.dram_tensor("wqk", [C, 2 * C], BF16, kind="ExternalInput")
    qkb_ext = nc.dram_tensor("qkb", [128, 8], F32, kind="ExternalInput")
    wv_ext = nc.dram_tensor("wv", [C + 1, C], BF16, kind="ExternalInput")
    pw_ext = nc.dram_tensor("pw", [C, C], BF16, kind="ExternalInput")
    pb_ext = nc.dram_tensor("pb", [128, 4], F32, kind="ExternalInput")
    gnw_ext = nc.dram_tensor("gnw", [128, 4], F32, kind="ExternalInput")
    gnb_ext = nc.dram_tensor("gnb", [128, 4], F32, kind="ExternalInput")
    out_ext = nc.dram_tensor("out", [C, T], BF16, kind="ExternalOutput")

    gfwd_np, gbck_np = _indicator_consts()
    gfwd_dram = nc.inline_tensor(gfwd_np, "gfwd")
    gbck_dram = nc.inline_tensor(gbck_np, "gbck")

    with tile.TileContext(nc) as tc, ExitStack() as ctx:
        pers = ctx.enter_context(tc.tile_pool(name="pers", bufs=1))
        work = ctx.enter_context(tc.tile_pool(name="work", bufs=2))
        small = ctx.enter_context(tc.tile_pool(name="small", bufs=2))
        p_pool = ctx.enter_context(tc.tile_pool(name="pT", bufs=4))
        rb_pool = ctx.enter_context(tc.tile_pool(name="rb", bufs=2))
        o_pool = ctx.enter_context(tc.tile_pool(name="osb", bufs=4))

        # ---- persistent SBUF tensors ----
        x_sb = [pers.tile([128, T], F32, tag=f"x{i}", name=f"x{i}") for i in range(4)]
        xn_sb = [pers.tile([128, T], BF16, tag=f"xn{i}", name=f"xn{i}") for i in range(4)]
        aun_sb = [pers.tile([64, 512], F32, tag=f"au{i}", name=f"au{i}")
                  for i in range(16)]  # unnormalized AV, (pr, h%2, nt)
        den_sb = [pers.tile([1, 512], F32, tag=f"de{i}", name=f"de{i}")
                  for i in range(16)]  # softmax denominators at base partition 0
        qk_sb = [pers.tile([128, T], BF16, tag=f"qk{i}", name=f"qk{i}") for i in range(8)]
        v_sb = [pers.tile([128, 520], BF16, tag=f"v{i}", name=f"v{i}") for i in range(8)]
        a_sb = [pers.tile([128, T], BF16, tag=f"a{i}", name=f"a{i}") for i in range(4)]
        wqk_sb = [pers.tile([128, 2 * C], BF16, tag=f"wqk{i}", name=f"wqk{i}") for i in range(4)]
        wv_sb = [pers.tile([128, C], BF16, tag=f"wv{i}", name=f"wv{i}") for i in range(4)]
        wv4_sb = pers.tile([1, C], BF16, tag="wv4", name="wv4")
        pw_sb = [pers.tile([128, C], BF16, tag=f"pw{i}", name=f"pw{i}") for i in range(4)]
        pb_sb = pers.tile([128, 4], F32, tag="pb", name="pb")
        qkb_sb = pers.tile([128, 8], F32, tag="qkb", name="qkb")
        gnw_sb = pers.tile([128, 4], F32, tag="gnw", name="gnw")
        gnb_sb = pers.tile([128, 4], F32, tag="gnb", name="gnb")
        gfwd_sb = pers.tile([128, 4 * NG], F32, tag="gfwd", name="gfwd")
        gbck_sb = pers.tile([NG, C], F32, tag="gbck", name="gbck")
        ones_sb = pers.tile([1, T], BF16, tag="ones", name="ones")
        gst_sb = pers.tile([NG, 2], F32, tag="gst", name="gst")
        ab_sb = [pers.tile([128, 2], F32, tag=f"ab{i}", name=f"ab{i}") for i in range(4)]

        # ---- input DMAs, split across the two HWDGE queues (SP + ACT) ----
        for i in range(4):
            for h2 in range(2):
                eng = nc.sync if (2 * i + h2) % 2 == 0 else nc.scalar
                eng.dma_start(x_sb[i][:, 512 * h2:512 * (h2 + 1)],
                              x_ext.ap()[128 * i:128 * (i + 1),
                                         512 * h2:512 * (h2 + 1)])
        nc.sync.dma_start(gfwd_sb[:], gfwd_dram.ap())
        nc.scalar.dma_start(gbck_sb[:], gbck_dram.ap())
        nc.scalar.dma_start(gnw_sb[:], gnw_ext.ap())
        nc.scalar.dma_start(gnb_sb[:], gnb_ext.ap())
        nc.scalar.dma_start(qkb_sb[:], qkb_ext.ap())
        for i in range(4):
            eng = nc.sync if i % 2 == 0 else nc.scalar
            eng.dma_start(wqk_sb[i][:], wqk_ext.ap()[128 * i:128 * (i + 1), :])
        for i in range(4):
            eng = nc.scalar if i % 2 == 0 else nc.sync
            eng.dma_start(wv_sb[i][:], wv_ext.ap()[128 * i:128 * (i + 1), :])
        nc.sync.dma_start(wv4_sb[:], wv_ext.ap()[C:C + 1, :])
        nc.scalar.dma_start(pb_sb[:], pb_ext.ap())
        for i in range(4):
            eng = nc.sync if i % 2 == 0 else nc.scalar
            eng.dma_start(pw_sb[i][:], pw_ext.ap()[128 * i:128 * (i + 1), :])
        nc.vector.memset(ones_sb[:], 1.0)

        # ---- GroupNorm statistics ----
        with tc.tile_pool(name="ps_misc", bufs=1, space="PSUM") as ps_misc:
            # PE warm-up: the HAM clock gate needs ~3.4us of activity to
            # unthrottle 1.2 -> 2.4 GHz. The PE would otherwise sit idle
            # during the input-DMA wait and every early matmul would run
            # cold. Dummy N=512 matmuls (dep-free after the ones memset)
            # keep it busy until real work arrives.
            ps_warm = ps_misc.tile([1, 512], F32, tag="warm", name="warm")
            for w in range(20):
                nc.tensor.matmul(ps_warm[:, :], ones_sb[:, 0:1],
                                 ones_sb[:, 0:512], start=True, stop=True)
            ps_g = ps_misc.tile([NG, 2], F32, tag="g", name="g")
            for i in range(4):
                bnst = small.tile([128, 12], F32, tag="bnst", name="bnst")
                nc.vector.bn_stats(bnst[:, 0:6], x_sb[i][:, 0:512])
                nc.vector.bn_stats(bnst[:, 6:12], x_sb[i][:, 512:1024])
                aggr = small.tile([128, 2], F32, tag="aggr", name="aggr")
                nc.vector.bn_aggr(aggr[:], bnst[:])
                st2 = small.tile([128, 2], F32, tag="st2", name="st2")
                nc.vector.tensor_copy(st2[:, 0:1], aggr[:, 0:1])
                # E[x^2] = mean^2 + var
                nc.vector.scalar_tensor_tensor(
                    st2[:, 1:2], aggr[:, 0:1], aggr[:, 0:1], aggr[:, 1:2],
                    op0=OP.mult, op1=OP.add)
                nc.tensor.matmul(ps_g[:, :], gfwd_sb[:, NG * i:NG * (i + 1)],
                                 st2[:, :], start=(i == 0), stop=(i == 3))

            # group var = E[x^2] - mean^2 ;  grs = rsqrt(var + eps)
            gsb = small.tile([NG, 2], F32, tag="gsb", name="gsb")
            nc.vector.tensor_copy(gsb[:], ps_g[:, :])
            gvar = small.tile([NG, 1], F32, tag="gvar", name="gvar")
            nc.vector.scalar_tensor_tensor(
                gvar[:], gsb[:, 0:1], gsb[:, 0:1], gsb[:, 1:2],
                op0=OP.mult, op1=OP.subtract)  # mean^2 - E[x^2] = -var
            nc.vector.tensor_scalar(gvar[:], gvar[:], -1.0, EPS,
                                    op0=OP.mult, op1=OP.add)  # var + eps
            r = small.tile([NG, 1], F32, tag="r", name="r")
            nc.vector.reciprocal(r[:], gvar[:])
            y = small.tile([NG, 1], F32, tag="y", name="y")
            nc.vector.tensor_scalar(y[:], r[:], 0.5, 0.5, op0=OP.mult, op1=OP.add)
            for _ in range(3):  # Newton for sqrt(r): y = 0.5*(y + r/y)
                q = small.tile([NG, 1], F32, tag="q", name="q")
                nc.vector.reciprocal(q[:], y[:])
                t = small.tile([NG, 1], F32, tag="t", name="t")
                nc.vector.tensor_mul(t[:], r[:], q[:])
                y2 = small.tile([NG, 1], F32, tag="y", name="y")
                nc.vector.tensor_add(y2[:], y[:], t[:])
                nc.vector.tensor_scalar(y2[:], y2[:], 0.5, None, op0=OP.mult)
                y = y2
            nc.vector.tensor_copy(gst_sb[:, 0:1], gsb[:, 0:1])
            nc.vector.tensor_copy(gst_sb[:, 1:2], y[:])

        with tc.tile_pool(name="ps_bc", bufs=2, space="PSUM") as ps_bcp:
            for i in range(4):
                ps_bc = ps_bcp.tile([128, 2], F32, tag="bc", name="bc")
                nc.tensor.matmul(ps_bc[:, :], gbck_sb[:, 128 * i:128 * (i + 1)],
                                 gst_sb[:, :], start=True, stop=True)
                # A = gnw * grs ; B = gnb - gmean * A
                nc.vector.tensor_mul(ab_sb[i][:, 0:1], gnw_sb[:, i:i + 1],
                                     ps_bc[:, 1:2])
                tmp = small.tile([128, 1], F32, tag="tmp", name="tmp")
                nc.vector.tensor_mul(tmp[:], ps_bc[:, 0:1], ab_sb[i][:, 0:1])
                nc.vector.tensor_sub(ab_sb[i][:, 1:2], gnb_sb[:, i:i + 1], tmp[:])
                # xn = x*A + B   (bf16)
                nc.vector.tensor_scalar(xn_sb[i][:], x_sb[i][:],
                                        ab_sb[i][:, 0:1], ab_sb[i][:, 1:2],
                                        op0=OP.mult, op1=OP.add)

        # ---- attention + background-fed v / qk ----
        with tc.tile_pool(name="ps_aux", bufs=2, space="PSUM") as ps_auxp, \
             tc.tile_pool(name="ps_s", bufs=2, space="PSUM") as ps_sp, \
             tc.tile_pool(name="ps_a", bufs=1, space="PSUM") as ps_ap:

            def v_gen(mt):
                # v^T chunk [t=128, c] with ones column -> v_sb[mt]
                ps_v = ps_auxp.tile([128, C], F32, tag="aux", name="ps_v")
                for kt in range(4):
                    nc.tensor.matmul(ps_v[:, :],
                                     xn_sb[kt][:, 128 * mt:128 * (mt + 1)],
                                     wv_sb[kt][:, :], start=(kt == 0), stop=False)
                    yield
                nc.tensor.matmul(ps_v[:, :], ones_sb[:, 0:128], wv4_sb[:, :],
                                 start=False, stop=True)
                nc.vector.tensor_copy(
                    v_sb[mt][:].rearrange("p (h w) -> p h w", w=CH + 1)[:, :, 0:CH],
                    ps_v[:].rearrange("p (h w) -> p h w", w=CH))
                nc.gpsimd.memset(
                    v_sb[mt][:].rearrange("p (h w) -> p h w", w=CH + 1)[:, :, CH:CH + 1],
                    1.0)
                yield

            def qk_gen(mt):
                # q or k row-block [128, T], bias added
                for nt in range(2):
                    ps_qk = ps_auxp.tile([128, 512], F32, tag="aux", name="ps_qk")
                    for kt in range(4):
                        nc.tensor.matmul(ps_qk[:, :],
                                         wqk_sb[kt][:, 128 * mt:128 * (mt + 1)],
                                         xn_sb[kt][:, 512 * nt:512 * (nt + 1)],
                                         start=(kt == 0), stop=(kt == 3))
                        yield
                    nc.vector.tensor_scalar(qk_sb[mt][:, 512 * nt:512 * (nt + 1)],
                                            ps_qk[:, :],
                                            qkb_sb[:, mt:mt + 1], None, op0=OP.add)
                    yield

            def emit_evac(h, ps_a, nt):
                # evacuate unnormalized AV + denom row to SBUF — frees the
                # PSUM bank fast so the next AV group isn't blocked. Denom
                # goes to base partition 0 (reciprocal_approx_fast needs it).
                idx = 4 * (h // 2) + 2 * (h % 2) + nt
                nc.vector.tensor_copy(aun_sb[idx][:], ps_a[0:CH, :])
                nc.vector.tensor_copy(den_sb[idx][:], ps_a[CH:CH + 1, :])

            def emit_norm(h, nt):
                # off-critical-path normalize: fast reciprocal, gpsimd
                # partition broadcast, DVE multiply
                idx = 4 * (h // 2) + 2 * (h % 2) + nt
                recip = small.tile([1, 512], F32, tag="recip", name="recip")
                nc.vector.reciprocal_approx_fast(recip[:], den_sb[idx][:])
                rb = rb_pool.tile([CH, 512], F32, tag="rb", name="rb")
                nc.gpsimd.partition_broadcast(rb[:], recip[:])
                dst = a_sb[h // 2][64 * (h % 2):64 * (h % 2) + 64,
                                   512 * nt:512 * (nt + 1)]
                nc.vector.tensor_mul(dst, aun_sb[idx][:], rb[:])

            # background feed: v first (AV of pair 0 needs it), then q,k for
            # pairs 1-3. Consumption rate decays as attention reaches ACT-bound
            # steady state.
            import itertools
            bg = itertools.chain(
                *[v_gen(mt) for mt in range(8)],
                qk_gen(1), qk_gen(5), qk_gen(2), qk_gen(6),
                qk_gen(3), qk_gen(7))

            for _ in qk_gen(0):
                pass
            for _ in qk_gen(4):
                pass

            pending = [None]  # deferred AV emitter (software pipeline depth 1)

            def flush():
                if pending[0] is not None:
                    pending[0]()
                    pending[0] = None

            gstep = 0
            for pr in range(4):
                hA, hB = 2 * pr, 2 * pr + 1
                qA, kA = qk_sb[pr], qk_sb[4 + pr]
                for nt in range(2):
                    ps_aA = ps_ap.tile([128, 512], F32, tag="aA", name="aA")
                    ps_aB = ps_ap.tile([128, 512], F32, tag="aB", name="aB")
                    for st in range(8):
                        ps_s = ps_sp.tile([128, T], F32, tag="s", name="s")
                        # two heads run CONCURRENTLY in the PE row halves:
                        # head A in rows 0-63, head B in rows 64-127 via
                        # tile_position (64, 0) — ~3x faster than serial.
                        nc.tensor.matmul(ps_s[:, 0:512],
                                         kA[0:64, 128 * st:128 * (st + 1)],
                                         qA[0:64, 512 * nt:512 * (nt + 1)],
                                         start=True, stop=True)
                        nc.tensor.matmul(ps_s[:, 512:1024],
                                         kA[64:128, 128 * st:128 * (st + 1)],
                                         qA[64:128, 512 * nt:512 * (nt + 1)],
                                         start=True, stop=True,
                                         tile_position=(64, 0))
                        pT = p_pool.tile([128, T], BF16, tag="pT", name="pT")
                        nc.scalar.activation(pT[:], ps_s[:], AF.Exp, scale=0.125)
                        flush()
                        rate = 5 if gstep < 8 else (3 if gstep < 16 else 2)
                        for _ in range(rate):
                            next(bg, None)
                        gstep += 1

                        def mk(aA=ps_aA, aB=ps_aB, p=pT, s=st, hA=hA, hB=hB,
                               nt=nt):
                            def emit():
                                nc.tensor.matmul(
                                    aA[0:CH + 1, :],
                                    v_sb[s][:, (CH + 1) * hA:(CH + 1) * (hA + 1)],
                                    p[:, 0:512],
                                    start=(s == 0), stop=(s == 7))
                                nc.tensor.matmul(
                                    aB[0:CH + 1, :],
                                    v_sb[s][:, (CH + 1) * hB:(CH + 1) * (hB + 1)],
                                    p[:, 512:1024],
                                    start=(s == 0), stop=(s == 7))
                                if s == 7:
                                    emit_evac(hA, aA, nt)
                                    emit_evac(hB, aB, nt)
                                    emit_norm(hA, nt)
                                    emit_norm(hB, nt)
                            return emit

                        pending[0] = mk()
            flush()
            for _ in bg:  # any feed leftovers
                pass

            # ---- proj + residual, reusing the aux PSUM pool (no pool
            # transition barrier between attention and proj). nt outer so the
            # nt=0 half overlaps the last pair's nt=1 normalize chain. ----
            for nt in range(2):
                for ot in range(4):
                    ps_h = ps_auxp.tile([128, C], F32, tag="aux", name="ps_h")
                    for kt in range(4):
                        nc.tensor.matmul(ps_h[:, :],
                                         pw_sb[kt][:, 128 * ot:128 * (ot + 1)],
                                         a_sb[kt][:, 512 * nt:512 * (nt + 1)],
                                         start=(kt == 0), stop=(kt == 3))
                    out_t = o_pool.tile([128, 512], BF16, tag="osb", name="osb")
                    nc.vector.scalar_tensor_tensor(
                        out_t[:], ps_h[:, :], pb_sb[:, ot:ot + 1],
                        x_sb[ot][:, 512 * nt:512 * (nt + 1)],
                        op0=OP.add, op1=OP.add)
                    eng = nc.sync if (2 * ot + nt) % 2 == 0 else nc.scalar
                    eng.dma_start(
                        out_ext.ap()[128 * ot:128 * (ot + 1),
                                     512 * nt:512 * (nt + 1)],
                        out_t[:])

    nc.compile()
    return nc


def _prep_host(x, gn_w, gn_b, qkv_w, qkv_b, proj_w, proj_b):
    s2 = 1.0 / math.sqrt(2.0)
    x2 = (x.reshape(B, C, T) * s2).astype(np.float32)

    w3 = np.asarray(qkv_w, np.float32).reshape(NH, 3, CH, C)
    b3 = np.asarray(qkv_b, np.float32).reshape(NH, 3, CH)
    qw = w3[:, 0].reshape(C, C)
    kw = w3[:, 1].reshape(C, C)
    vw = w3[:, 2].reshape(C, C)
    qb = b3[:, 0].reshape(C)
    kb = b3[:, 1].reshape(C)
    vb = b3[:, 2].reshape(C)

    wqk = np.concatenate([qw, kw], 0).T.astype(ml_dtypes.bfloat16)  # [512, 1024]
    qkb = np.concatenate([qb, kb]).reshape(8, 128).T.astype(np.float32).copy()
    wv = np.concatenate([vw.T, vb[None, :]], 0).astype(ml_dtypes.bfloat16)
    pw = (np.asarray(proj_w, np.float32).T * s2).astype(ml_dtypes.bfloat16)
    pb = (np.asarray(proj_b, np.float32) * s2).reshape(4, 128).T.copy()
    gnw_t = np.asarray(gn_w, np.float32).reshape(4, 128).T.copy()
    gnb_t = np.asarray(gn_b, np.float32).reshape(4, 128).T.copy()

    base = {"wqk": np.ascontiguousarray(wqk), "qkb": qkb,
            "wv": np.ascontiguousarray(wv), "pw": np.ascontiguousarray(pw),
            "pb": pb, "gnw": gnw_t, "gnb": gnb_t}
    return x2, base


def kernel(x, gn_w, gn_b, qkv_w, qkv_b, proj_w, proj_b):
    x2, base = _prep_host(x, gn_w, gn_b, qkv_w, qkv_b, proj_w, proj_b)
    if "nc" not in _CACHED:
        _CACHED["nc"] = build_graph()
    nc = _CACHED["nc"]
    in_maps = [dict(base, x=np.ascontiguousarray(x2[i])) for i in range(N_CORES)]
    res = run_bass_kernel_spmd(nc, in_maps, core_ids=list(range(N_CORES)))
    out = np.stack([res.results[i]["out"].astype(np.float32)
                    for i in range(N_CORES)], 0)
    return out.reshape(B, C, 32, 32)
